# revision 16
# baseline (speedup 1.0000x reference)
"""GNN message passing (segment_sum of gathered node features) on 8 TRN2 cores.

Default strategy (KMODE=mm, one-hot count-matrix matmul):
  h[r] = sum_{e: row_e == r} x[col_e]  ==  h = A @ x  with A[r, c] = #edges c->r.
- Destinations are sharded across the 8 cores (1280 dst nodes per core, 10240
  padded).  Per core, h_c^T = x^T @ A_c^T is computed as a sum over 80 source
  tiles of 128 nodes: for each source tile s,
      psum[f, d] += X_s[src, f]^T @ M_s[src, d]
  where X_s ([128 src, 128 feat] bf16) is the matmul stationary and
  M_s ([128 src, 1280 dst] fp8 e4m3, exact small integer counts) is the moving
  operand streamed from DRAM.  PSUM accumulates over all 80 source tiles
  (3 bank regions of 512/512/256 fp32 columns); the result is DMAed out
  transposed and fixed up on host.
- No per-edge DMA descriptors at all: the only HBM traffic is the contiguous
  fp8 count matrix (12.8 MB/core), x (2.6 MB replicated), and the output.
  PE does 80x3 matmuls of K=128, N<=512 per core (~43 us) overlapping the
  M-matrix stream.

Fallback strategy (KMODE=gather): node-sharded CSR dma_gather + DVE tree
reduction (see _build_program_gather) — bandwidth-bound at ~256B/edge.
"""

import os
from contextlib import ExitStack, nullcontext

import numpy as np

N = 10000
F = 128
E = 640000
NCORES = 8
P = 128

# mm-mode geometry
ST = 80                # source tiles of 128 nodes
NPAD = ST * P          # 10240 padded nodes (sources)
DLOC = N // NCORES     # 1250 destination nodes per core (exact, no padding)

# gather-mode geometry
NT = 80
S = NT // NCORES

_PROG_CACHE = {}


def _gdtype():
    return os.environ.get("KDTYPE", "f32")


# ---------------------------------------------------------------- mm mode ---


def _prep_mm(x, edge_index):
    import ml_dtypes

    x = np.ascontiguousarray(np.asarray(x, dtype=np.float32))
    ei = np.asarray(edge_index)
    row = ei[0].astype(np.int64)  # destination
    col = ei[1].astype(np.int64)  # source

    xpad = np.zeros((NPAD, F), np.float32)
    xpad[:N] = x
    # stationary layout: xsb[p, s, f] = x[s*128 + p, f]
    xsb = np.ascontiguousarray(
        xpad.reshape(ST, P, F).transpose(1, 0, 2)
    ).astype(ml_dtypes.bfloat16)

    msbs = []
    for c in range(NCORES):
        mask = (row >= c * DLOC) & (row < (c + 1) * DLOC)
        Mc = np.zeros((NPAD, DLOC), np.uint8)
        np.add.at(Mc, (col[mask], row[mask] - c * DLOC), 1)
        assert Mc.max() <= 16, "edge multiplicity too large for exact fp8"
        # moving layout: msb[p, s, d] = Mc[s*128 + p, d]
        msb = np.ascontiguousarray(
            Mc.reshape(ST, P, DLOC).transpose(1, 0, 2)
        ).astype(ml_dtypes.float8_e4m3)
        msbs.append(msb)
    return xsb, msbs


def _build_program_mm():
    import concourse.bass as bass  # noqa: F401
    import concourse.tile as tile
    from concourse import bacc, mybir

    nc = bacc.Bacc(
        "TRN2",
        target_bir_lowering=False,
        debug=False,
        num_devices=NCORES,
    )
    xp = nc.declare_dram_parameter("xp", [P, ST, F], mybir.dt.bfloat16, isOutput=False)
    m = nc.declare_dram_parameter("m", [P, ST, DLOC], mybir.dt.float8e4, isOutput=False)
    out = nc.declare_dram_parameter("out", [P, DLOC], mybir.dt.float32, isOutput=True)

    G = int(os.environ.get("KGROUP", "8"))     # source tiles per M-stream chunk
    _BUFS = int(os.environ.get("KBUFS", "4"))  # M-chunk buffers in flight
    _REPS = int(os.environ.get("KREPS", "1"))
    NGRP = ST // G
    # psum column regions (each within one 2KB bank)
    spans = [(j * 512, min((j + 1) * 512, DLOC)) for j in range((DLOC + 511) // 512)]

    with tile.TileContext(nc) as tc:
        with ExitStack() as ctx:
            xpool = ctx.enter_context(tc.tile_pool(name="xs", bufs=1))
            mpool = ctx.enter_context(tc.tile_pool(name="mchunk", bufs=_BUFS))
            pspool = ctx.enter_context(tc.tile_pool(name="acc", bufs=1, space="PSUM"))

            xs = xpool.tile([P, ST, F], mybir.dt.bfloat16)
            if _REPS > 1:
                nc.sync.dma_start(xs[:], xp[:, :, :])
            opool = ctx.enter_context(tc.tile_pool(name="ostage", bufs=1))

            ps = [
                pspool.tile(
                    [P, n1 - n0], mybir.dt.float32, tag=f"ps{j}", name=f"ps{j}"
                )
                for j, (n0, n1) in enumerate(spans)
            ]

            # HAM warm-up: dummy matmuls keep the PE busy during the initial
            # DMA fill so the real matmul stream starts at the warm clock
            WARM = int(os.environ.get("KWARM", "14"))
            if WARM and _REPS == 1:
                wz = xpool.tile([P, 640], mybir.dt.bfloat16, name="warmz")
                nc.any.memzero(wz[:])
                wps = pspool.tile([P, 512], mybir.dt.float32, name="warmps")
                for _ in range(WARM):
                    nc.tensor.matmul(
                        wps[:], lhsT=wz[:, :128], rhs=wz[:, 128:640],
                        start=True, stop=True,
                    )

            loop_ctx = tc.For_i(0, _REPS, 1) if _REPS > 1 else nullcontext()
            with loop_ctx:
                for g in range(NGRP):
                    if _REPS == 1:
                        # interleave the x chunk loads with the M stream so the
                        # first matmuls start after ~one chunk instead of the
                        # whole 2.6MB x load
                        nc.sync.dma_start(
                            xs[:, g * G : (g + 1) * G, :], xp[:, g * G : (g + 1) * G, :]
                        )
                    mt = mpool.tile([P, G, DLOC], mybir.dt.float8e4, tag="m")
                    nc.sync.dma_start(mt[:], m[:, g * G : (g + 1) * G, :])
                    for k in range(G):
                        s = g * G + k
                        for j, (n0, n1) in enumerate(spans):
                            nc.tensor.matmul(
                                ps[j][:, :],
                                lhsT=xs[:, s, :],
                                rhs=mt[:, k, n0:n1],
                                start=(s == 0),
                                stop=(s == ST - 1),
                            )
                ostage = opool.tile([P, DLOC], mybir.dt.float32, tag="ostage")
                for j, (n0, n1) in enumerate(spans):
                    nc.vector.tensor_copy(ostage[:, n0:n1], ps[j][:, :])
                nc.sync.dma_start(out[:, :], ostage[:, :])

    nc.finalize()
    return nc


def _run_mm(x, edge_index):
    from concourse.bass_utils import run_bass_kernel_spmd

    xsb, msbs = _prep_mm(x, edge_index)

    key = (
        "mm",
        tuple(os.environ.get(k, "") for k in ("KGROUP", "KBUFS", "KREPS")),
    )
    if key not in _PROG_CACHE:
        _PROG_CACHE[key] = _build_program_mm()
    nc = _PROG_CACHE[key]

    in_maps = [{"xp": xsb, "m": msbs[c]} for c in range(NCORES)]
    res = run_bass_kernel_spmd(nc, in_maps, list(range(NCORES)))
    global LAST_RESULT
    LAST_RESULT = res

    h = np.zeros((N, F), dtype=np.float32)
    for c in range(NCORES):
        oc = np.asarray(res.results[c]["out"])  # [F, DLOC]
        lo = c * DLOC
        hi = min((c + 1) * DLOC, N)
        if hi > lo:
            h[lo:hi] = oc[:, : hi - lo].T
    return h


# ------------------------------------------------------------ gather mode ---


def _prep_gather(x, edge_index):
    x = np.ascontiguousarray(np.asarray(x, dtype=np.float32))
    ei = np.asarray(edge_index)
    row = ei[0].astype(np.int64)
    col = ei[1].astype(np.int64)

    deg = np.bincount(row, minlength=N)

    # nodes ordered by degree desc; stable for reproducibility
    order = np.argsort(-deg, kind="stable")
    order_pad = np.full(NT * P, -1, dtype=np.int64)
    order_pad[:N] = order

    # CSR of incoming neighbors, grouped by destination row
    eorder = np.argsort(row, kind="stable")
    scol = col[eorder].astype(np.int16)
    indptr = np.zeros(N + 1, dtype=np.int64)
    indptr[1:] = np.cumsum(deg)

    # slot max degrees: tiles 8s..8s+7 in slot s; degrees are non-increasing
    # along order_pad so the first node of tile 8s has the slot max.
    Ks = []
    degs_sorted = np.zeros(NT * P, dtype=np.int64)
    degs_sorted[:N] = deg[order]
    for s in range(S):
        Ks.append(max(int(degs_sorted[(s * NCORES) * P]), 1))

    # x with an extra zero row used by padding indices
    if _gdtype() == "bf16":
        import ml_dtypes

        xpad = np.zeros((N + 1, F), dtype=ml_dtypes.bfloat16)
        xpad[:N] = x.astype(ml_dtypes.bfloat16)
    else:
        xpad = np.zeros((N + 1, F), dtype=np.float32)
        xpad[:N] = x

    # per-core wrapped index tensors
    idx_cores = []
    for c in range(NCORES):
        blocks = []
        for s in range(S):
            K = Ks[s]
            t = s * NCORES + c
            blk = np.full((K, P), N, dtype=np.int16)
            for p in range(P):
                v = order_pad[t * P + p]
                if v >= 0:
                    d0, d1 = indptr[v], indptr[v + 1]
                    if d1 > d0:
                        blk[: d1 - d0, p] = scol[d0:d1]
            idx_lin = blk.reshape(-1)            # i = j*128 + p
            w = idx_lin.reshape(-1, 16)          # [n/16, 16]
            sb = np.tile(w.T, (8, 1))            # [128, n/16] replicated x8
            blocks.append(sb)
        idx_cores.append(np.ascontiguousarray(np.concatenate(blocks, axis=1)))

    return xpad, idx_cores, Ks, order_pad


def _build_program_gather(Ks):
    import concourse.bass as bass  # noqa: F401
    import concourse.tile as tile
    from concourse import bacc, mybir

    s_total = sum(K * P // 16 for K in Ks)

    nc = bacc.Bacc(
        "TRN2",
        target_bir_lowering=False,
        debug=False,
        num_devices=NCORES,
        dynamic_dma_scratch_size=int(os.environ.get("KSCRATCH", "98304")),
        num_swdge_queues=int(os.environ.get("KNQ", "4")),
    )
    gdt = mybir.dt.bfloat16 if _gdtype() == "bf16" else mybir.dt.float32
    xp = nc.declare_dram_parameter("xp", [N + 1, F], gdt, isOutput=False)
    idx = nc.declare_dram_parameter("idx", [128, s_total], mybir.dt.int16, isOutput=False)
    out = nc.declare_dram_parameter("out", [S * P, F], mybir.dt.float32, isOutput=True)

    CHUNK = int(os.environ.get("KCHUNK", "12"))  # blocks of 128 idx per dma_gather

    with tile.TileContext(nc) as tc:
        with ExitStack() as ctx:
            _BUFS = int(os.environ.get("KBUFS", "10"))
            _REPS = int(os.environ.get("KREPS", "1"))
            ipool = ctx.enter_context(tc.tile_pool(name="idx", bufs=1))
            gpool = ctx.enter_context(tc.tile_pool(name="gath", bufs=_BUFS))
            apool = ctx.enter_context(tc.tile_pool(name="acc", bufs=min(_BUFS, 2)))

            # load the whole wrapped index array into SBUF once
            it_all = ipool.tile([128, s_total], mybir.dt.int16)
            nc.sync.dma_start(it_all[:], idx[:, :])

            _NQ = int(os.environ.get("KNQ", "4"))
            _gq = [0]

            loop_ctx = tc.For_i(0, _REPS, 1) if _REPS > 1 else nullcontext()
            with loop_ctx:
                off = 0  # col offset into idx (wrapped: block j -> cols j*8..j*8+7)
                for s in range(S):
                    K = Ks[s]
                    acc = apool.tile([128, F], mybir.dt.float32, tag="acc")
                    nchunks = (K + CHUNK - 1) // CHUNK
                    # equalize chunk sizes (avoid tiny tail gathers)
                    base, extra = divmod(K, nchunks)
                    bounds = [0]
                    for c in range(nchunks):
                        bounds.append(bounds[-1] + base + (1 if c < extra else 0))
                    for c in range(nchunks):
                        j0, j1 = bounds[c], bounds[c + 1]
                        W = j1 - j0
                        n = W * P
                        g = gpool.tile([128, n], gdt, tag="g")
                        nc.gpsimd.dma_gather(
                            g[:].rearrange("p (k f) -> p k f", f=F),
                            xp[:, :],
                            it_all[:, off + j0 * 8 : off + j1 * 8],
                            num_idxs=n,
                            num_idxs_reg=n,
                            elem_size=F,
                            single_packet=False,
                            queue_num=_gq[0],
                        )
                        _gq[0] = (_gq[0] + 1) % _NQ

                        # in-place tree reduction over the W feature blocks
                        while W > 1:
                            half = W // 2
                            nc.vector.tensor_add(
                                out=g[:, 0 : half * F],
                                in0=g[:, 0 : half * F],
                                in1=g[:, (W - half) * F : W * F],
                            )
                            W -= half
                        red = g
                        if c == 0:
                            nc.vector.tensor_copy(acc[:], red[:, 0:F])
                        else:
                            nc.vector.tensor_add(out=acc[:], in0=acc[:], in1=red[:, 0:F])

                    nc.sync.dma_start(out[s * P : (s + 1) * P, :], acc[:])
                    off += K * 8

    nc.finalize()
    return nc


def _run_gather(x, edge_index):
    from concourse.bass_utils import run_bass_kernel_spmd

    xpad, idx_cores, Ks, order_pad = _prep_gather(x, edge_index)

    key = (
        "gather",
        tuple(Ks),
        tuple(
            os.environ.get(k, "")
            for k in ("KCHUNK", "KBUFS", "KREPS", "KSCRATCH", "KNQ", "KDTYPE")
        ),
    )
    if key not in _PROG_CACHE:
        _PROG_CACHE[key] = _build_program_gather(Ks)
    nc = _PROG_CACHE[key]

    in_maps = [{"xp": xpad, "idx": idx_cores[c]} for c in range(NCORES)]
    try:
        res = run_bass_kernel_spmd(nc, in_maps, list(range(NCORES)))
    except Exception:
        # fall back to a conservative configuration (smaller gathers,
        # shallow pipelining) in case the tuned one trips the SWDGE ring
        os.environ["KCHUNK"] = "8"
        os.environ["KBUFS"] = "2"
        os.environ["KNQ"] = "1"
        os.environ["KSCRATCH"] = "16384"
        nc = _build_program_gather(Ks)
        res = run_bass_kernel_spmd(nc, in_maps, list(range(NCORES)))
    global LAST_RESULT
    LAST_RESULT = res

    h = np.zeros((N, F), dtype=np.float32)
    for c in range(NCORES):
        oc = np.asarray(res.results[c]["out"]).astype(np.float32)
        for s in range(S):
            t = s * NCORES + c
            vs = order_pad[t * P : (t + 1) * P]
            valid = vs >= 0
            if valid.any():
                h[vs[valid]] = oc[s * P : (s + 1) * P][valid]
    return h


def kernel(x, edge_index):
    mode = os.environ.get("KMODE", "mm")
    if mode == "gather":
        return _run_gather(x, edge_index)
    try:
        return _run_mm(x, edge_index)
    except Exception:
        os.environ["KDTYPE"] = "bf16"
        return _run_gather(x, edge_index)


# revision 19
# speedup vs baseline: 1.0001x; 1.0001x over previous
"""GNN message passing (segment_sum of gathered node features) on 8 TRN2 cores.

Default strategy (KMODE=mm, one-hot count-matrix matmul):
  h[r] = sum_{e: row_e == r} x[col_e]  ==  h = A @ x  with A[r, c] = #edges c->r.
- Destinations are sharded across the 8 cores (1250 dst nodes per core, no
  padding).  Per core, h_c^T = x^T @ A_c^T is computed as a sum over 80 source
  tiles of 128 nodes: for each source tile s,
      psum[f, d] += X_s[src, f]^T @ M_s[src, d]
  where X_s ([128 src, 128 feat] bf16) is the matmul stationary and
  M_s ([128 src, 1250 dst] fp8 e4m3, exact small integer counts) is the moving
  operand streamed from DRAM in 8-src-tile chunks.  PSUM accumulates over all
  80 source tiles (3 bank regions of 512/512/226 fp32 columns); the result is
  DMAed out transposed and fixed up on host.
- No per-edge DMA descriptors at all: the only HBM traffic is the contiguous
  fp8 count matrix (12.8 MB/core), x (2.6 MB replicated), and the output.
  PE does 80x3 matmuls of K=128, N<=512 per core overlapping the M stream;
  dummy warm-up matmuls during the initial DMA fill keep the HAM clock warm.
  Measured ~57-62 us/iter on HW vs 492 us for the fp32 gather baseline.

Fallback strategy (KMODE=gather): node-sharded CSR dma_gather + DVE tree
reduction (see _build_program_gather) — bandwidth-bound at ~256B/edge.
"""

import os
from contextlib import ExitStack, nullcontext

import numpy as np

N = 10000
F = 128
E = 640000
NCORES = 8
P = 128

# mm-mode geometry
ST = 80                # source tiles of 128 nodes
NPAD = ST * P          # 10240 padded nodes (sources)
DLOC = N // NCORES     # 1250 destination nodes per core (exact, no padding)

# gather-mode geometry
NT = 80
S = NT // NCORES

_PROG_CACHE = {}


def _gdtype():
    return os.environ.get("KDTYPE", "f32")


# ---------------------------------------------------------------- mm mode ---


def _prep_mm(x, edge_index):
    import ml_dtypes

    x = np.ascontiguousarray(np.asarray(x, dtype=np.float32))
    ei = np.asarray(edge_index)
    row = ei[0].astype(np.int64)  # destination
    col = ei[1].astype(np.int64)  # source

    xpad = np.zeros((NPAD, F), np.float32)
    xpad[:N] = x
    # stationary layout: xsb[p, s, f] = x[s*128 + p, f]
    xsb = np.ascontiguousarray(
        xpad.reshape(ST, P, F).transpose(1, 0, 2)
    ).astype(ml_dtypes.bfloat16)

    msbs = []
    for c in range(NCORES):
        mask = (row >= c * DLOC) & (row < (c + 1) * DLOC)
        Mc = np.zeros((NPAD, DLOC), np.uint8)
        np.add.at(Mc, (col[mask], row[mask] - c * DLOC), 1)
        assert Mc.max() <= 16, "edge multiplicity too large for exact fp8"
        # moving layout: msb[p, s, d] = Mc[s*128 + p, d]
        msb = np.ascontiguousarray(
            Mc.reshape(ST, P, DLOC).transpose(1, 0, 2)
        ).astype(ml_dtypes.float8_e4m3)
        msbs.append(msb)
    return xsb, msbs


def _build_program_mm():
    import concourse.bass as bass  # noqa: F401
    import concourse.tile as tile
    from concourse import bacc, mybir

    nc = bacc.Bacc(
        "TRN2",
        target_bir_lowering=False,
        debug=False,
        num_devices=NCORES,
    )
    xp = nc.declare_dram_parameter("xp", [P, ST, F], mybir.dt.bfloat16, isOutput=False)
    m = nc.declare_dram_parameter("m", [P, ST, DLOC], mybir.dt.float8e4, isOutput=False)
    out = nc.declare_dram_parameter("out", [P, DLOC], mybir.dt.float32, isOutput=True)

    G = int(os.environ.get("KGROUP", "8"))     # source tiles per M-stream chunk
    _BUFS = int(os.environ.get("KBUFS", "4"))  # M-chunk buffers in flight
    _REPS = int(os.environ.get("KREPS", "1"))
    NGRP = ST // G
    # psum column regions (each within one 2KB bank)
    spans = [(j * 512, min((j + 1) * 512, DLOC)) for j in range((DLOC + 511) // 512)]

    with tile.TileContext(nc) as tc:
        with ExitStack() as ctx:
            xpool = ctx.enter_context(tc.tile_pool(name="xs", bufs=1))
            mpool = ctx.enter_context(tc.tile_pool(name="mchunk", bufs=_BUFS))
            pspool = ctx.enter_context(tc.tile_pool(name="acc", bufs=1, space="PSUM"))

            xs = xpool.tile([P, ST, F], mybir.dt.bfloat16)
            if _REPS > 1:
                nc.sync.dma_start(xs[:], xp[:, :, :])
            opool = ctx.enter_context(tc.tile_pool(name="ostage", bufs=1))

            ps = [
                pspool.tile(
                    [P, n1 - n0], mybir.dt.float32, tag=f"ps{j}", name=f"ps{j}"
                )
                for j, (n0, n1) in enumerate(spans)
            ]

            # HAM warm-up: dummy matmuls keep the PE busy during the initial
            # DMA fill so the real matmul stream starts at the warm clock
            WARM = int(os.environ.get("KWARM", "14"))
            if WARM and _REPS == 1:
                wz = xpool.tile([P, 640], mybir.dt.bfloat16, name="warmz")
                nc.any.memzero(wz[:])
                wps = pspool.tile([P, 512], mybir.dt.float32, name="warmps")
                for _ in range(WARM):
                    nc.tensor.matmul(
                        wps[:], lhsT=wz[:, :128], rhs=wz[:, 128:640],
                        start=True, stop=True,
                    )

            loop_ctx = tc.For_i(0, _REPS, 1) if _REPS > 1 else nullcontext()
            with loop_ctx:
                for g in range(NGRP):
                    if _REPS == 1:
                        # interleave the x chunk loads with the M stream so the
                        # first matmuls start after ~one chunk instead of the
                        # whole 2.6MB x load
                        nc.sync.dma_start(
                            xs[:, g * G : (g + 1) * G, :], xp[:, g * G : (g + 1) * G, :]
                        )
                    mt = mpool.tile([P, G, DLOC], mybir.dt.float8e4, tag="m")
                    nc.sync.dma_start(mt[:], m[:, g * G : (g + 1) * G, :])
                    for k in range(G):
                        s = g * G + k
                        for j, (n0, n1) in enumerate(spans):
                            nc.tensor.matmul(
                                ps[j][:, :],
                                lhsT=xs[:, s, :],
                                rhs=mt[:, k, n0:n1],
                                start=(s == 0),
                                stop=(s == ST - 1),
                            )
                ostage = opool.tile([P, DLOC], mybir.dt.float32, tag="ostage")
                for j, (n0, n1) in enumerate(spans):
                    nc.vector.tensor_copy(ostage[:, n0:n1], ps[j][:, :])
                nc.sync.dma_start(out[:, :], ostage[:, :])

    nc.finalize()
    return nc


def _run_mm(x, edge_index):
    from concourse.bass_utils import run_bass_kernel_spmd

    xsb, msbs = _prep_mm(x, edge_index)

    key = (
        "mm",
        tuple(os.environ.get(k, "") for k in ("KGROUP", "KBUFS", "KREPS", "KWARM")),
    )
    if key not in _PROG_CACHE:
        _PROG_CACHE[key] = _build_program_mm()
    nc = _PROG_CACHE[key]

    in_maps = [{"xp": xsb, "m": msbs[c]} for c in range(NCORES)]
    res = run_bass_kernel_spmd(nc, in_maps, list(range(NCORES)))
    global LAST_RESULT
    LAST_RESULT = res

    h = np.zeros((N, F), dtype=np.float32)
    for c in range(NCORES):
        oc = np.asarray(res.results[c]["out"])  # [F, DLOC]
        lo = c * DLOC
        hi = min((c + 1) * DLOC, N)
        if hi > lo:
            h[lo:hi] = oc[:, : hi - lo].T
    return h


# ------------------------------------------------------------ gather mode ---


def _prep_gather(x, edge_index):
    x = np.ascontiguousarray(np.asarray(x, dtype=np.float32))
    ei = np.asarray(edge_index)
    row = ei[0].astype(np.int64)
    col = ei[1].astype(np.int64)

    deg = np.bincount(row, minlength=N)

    # nodes ordered by degree desc; stable for reproducibility
    order = np.argsort(-deg, kind="stable")
    order_pad = np.full(NT * P, -1, dtype=np.int64)
    order_pad[:N] = order

    # CSR of incoming neighbors, grouped by destination row
    eorder = np.argsort(row, kind="stable")
    scol = col[eorder].astype(np.int16)
    indptr = np.zeros(N + 1, dtype=np.int64)
    indptr[1:] = np.cumsum(deg)

    # slot max degrees: tiles 8s..8s+7 in slot s; degrees are non-increasing
    # along order_pad so the first node of tile 8s has the slot max.
    Ks = []
    degs_sorted = np.zeros(NT * P, dtype=np.int64)
    degs_sorted[:N] = deg[order]
    for s in range(S):
        Ks.append(max(int(degs_sorted[(s * NCORES) * P]), 1))

    # x with an extra zero row used by padding indices
    if _gdtype() == "bf16":
        import ml_dtypes

        xpad = np.zeros((N + 1, F), dtype=ml_dtypes.bfloat16)
        xpad[:N] = x.astype(ml_dtypes.bfloat16)
    else:
        xpad = np.zeros((N + 1, F), dtype=np.float32)
        xpad[:N] = x

    # per-core wrapped index tensors
    idx_cores = []
    for c in range(NCORES):
        blocks = []
        for s in range(S):
            K = Ks[s]
            t = s * NCORES + c
            blk = np.full((K, P), N, dtype=np.int16)
            for p in range(P):
                v = order_pad[t * P + p]
                if v >= 0:
                    d0, d1 = indptr[v], indptr[v + 1]
                    if d1 > d0:
                        blk[: d1 - d0, p] = scol[d0:d1]
            idx_lin = blk.reshape(-1)            # i = j*128 + p
            w = idx_lin.reshape(-1, 16)          # [n/16, 16]
            sb = np.tile(w.T, (8, 1))            # [128, n/16] replicated x8
            blocks.append(sb)
        idx_cores.append(np.ascontiguousarray(np.concatenate(blocks, axis=1)))

    return xpad, idx_cores, Ks, order_pad


def _build_program_gather(Ks):
    import concourse.bass as bass  # noqa: F401
    import concourse.tile as tile
    from concourse import bacc, mybir

    s_total = sum(K * P // 16 for K in Ks)

    nc = bacc.Bacc(
        "TRN2",
        target_bir_lowering=False,
        debug=False,
        num_devices=NCORES,
        dynamic_dma_scratch_size=int(os.environ.get("KSCRATCH", "98304")),
        num_swdge_queues=int(os.environ.get("KNQ", "4")),
    )
    gdt = mybir.dt.bfloat16 if _gdtype() == "bf16" else mybir.dt.float32
    xp = nc.declare_dram_parameter("xp", [N + 1, F], gdt, isOutput=False)
    idx = nc.declare_dram_parameter("idx", [128, s_total], mybir.dt.int16, isOutput=False)
    out = nc.declare_dram_parameter("out", [S * P, F], mybir.dt.float32, isOutput=True)

    CHUNK = int(os.environ.get("KCHUNK", "12"))  # blocks of 128 idx per dma_gather

    with tile.TileContext(nc) as tc:
        with ExitStack() as ctx:
            _BUFS = int(os.environ.get("KBUFS", "10"))
            _REPS = int(os.environ.get("KREPS", "1"))
            ipool = ctx.enter_context(tc.tile_pool(name="idx", bufs=1))
            gpool = ctx.enter_context(tc.tile_pool(name="gath", bufs=_BUFS))
            apool = ctx.enter_context(tc.tile_pool(name="acc", bufs=min(_BUFS, 2)))

            # load the whole wrapped index array into SBUF once
            it_all = ipool.tile([128, s_total], mybir.dt.int16)
            nc.sync.dma_start(it_all[:], idx[:, :])

            _NQ = int(os.environ.get("KNQ", "4"))
            _gq = [0]

            loop_ctx = tc.For_i(0, _REPS, 1) if _REPS > 1 else nullcontext()
            with loop_ctx:
                off = 0  # col offset into idx (wrapped: block j -> cols j*8..j*8+7)
                for s in range(S):
                    K = Ks[s]
                    acc = apool.tile([128, F], mybir.dt.float32, tag="acc")
                    nchunks = (K + CHUNK - 1) // CHUNK
                    # equalize chunk sizes (avoid tiny tail gathers)
                    base, extra = divmod(K, nchunks)
                    bounds = [0]
                    for c in range(nchunks):
                        bounds.append(bounds[-1] + base + (1 if c < extra else 0))
                    for c in range(nchunks):
                        j0, j1 = bounds[c], bounds[c + 1]
                        W = j1 - j0
                        n = W * P
                        g = gpool.tile([128, n], gdt, tag="g")
                        nc.gpsimd.dma_gather(
                            g[:].rearrange("p (k f) -> p k f", f=F),
                            xp[:, :],
                            it_all[:, off + j0 * 8 : off + j1 * 8],
                            num_idxs=n,
                            num_idxs_reg=n,
                            elem_size=F,
                            single_packet=False,
                            queue_num=_gq[0],
                        )
                        _gq[0] = (_gq[0] + 1) % _NQ

                        # in-place tree reduction over the W feature blocks
                        while W > 1:
                            half = W // 2
                            nc.vector.tensor_add(
                                out=g[:, 0 : half * F],
                                in0=g[:, 0 : half * F],
                                in1=g[:, (W - half) * F : W * F],
                            )
                            W -= half
                        red = g
                        if c == 0:
                            nc.vector.tensor_copy(acc[:], red[:, 0:F])
                        else:
                            nc.vector.tensor_add(out=acc[:], in0=acc[:], in1=red[:, 0:F])

                    nc.sync.dma_start(out[s * P : (s + 1) * P, :], acc[:])
                    off += K * 8

    nc.finalize()
    return nc


def _run_gather(x, edge_index):
    from concourse.bass_utils import run_bass_kernel_spmd

    xpad, idx_cores, Ks, order_pad = _prep_gather(x, edge_index)

    key = (
        "gather",
        tuple(Ks),
        tuple(
            os.environ.get(k, "")
            for k in ("KCHUNK", "KBUFS", "KREPS", "KSCRATCH", "KNQ", "KDTYPE")
        ),
    )
    if key not in _PROG_CACHE:
        _PROG_CACHE[key] = _build_program_gather(Ks)
    nc = _PROG_CACHE[key]

    in_maps = [{"xp": xpad, "idx": idx_cores[c]} for c in range(NCORES)]
    try:
        res = run_bass_kernel_spmd(nc, in_maps, list(range(NCORES)))
    except Exception:
        # fall back to a conservative configuration (smaller gathers,
        # shallow pipelining) in case the tuned one trips the SWDGE ring
        os.environ["KCHUNK"] = "8"
        os.environ["KBUFS"] = "2"
        os.environ["KNQ"] = "1"
        os.environ["KSCRATCH"] = "16384"
        nc = _build_program_gather(Ks)
        res = run_bass_kernel_spmd(nc, in_maps, list(range(NCORES)))
    global LAST_RESULT
    LAST_RESULT = res

    h = np.zeros((N, F), dtype=np.float32)
    for c in range(NCORES):
        oc = np.asarray(res.results[c]["out"]).astype(np.float32)
        for s in range(S):
            t = s * NCORES + c
            vs = order_pad[t * P : (t + 1) * P]
            valid = vs >= 0
            if valid.any():
                h[vs[valid]] = oc[s * P : (s + 1) * P][valid]
    return h


def kernel(x, edge_index):
    mode = os.environ.get("KMODE", "mm")
    if mode == "gather":
        return _run_gather(x, edge_index)
    try:
        return _run_mm(x, edge_index)
    except Exception:
        # emergency path: fp32 CSR gather (slower but fully validated)
        return _run_gather(x, edge_index)


# revision 22
# speedup vs baseline: 1.0668x; 1.0667x over previous
"""GNN message passing (segment_sum of gathered node features) on 8 TRN2 cores.

Default strategy (KMODE=mm, one-hot count-matrix matmul):
  h[r] = sum_{e: row_e == r} x[col_e]  ==  h = A @ x  with A[r, c] = #edges c->r.
- Destinations are sharded across the 8 cores (1250 dst nodes per core, no
  padding).  Per core, h_c^T = x^T @ A_c^T is computed as a sum over 80 source
  tiles of 128 nodes: for each source tile s,
      psum[f, d] += X_s[src, f]^T @ M_s[src, d]
  where X_s ([128 src, 128 feat] bf16) is the matmul stationary and
  M_s ([128 src, 1250 dst] fp8 e4m3, exact small integer counts) is the moving
  operand streamed from DRAM in 8-src-tile chunks.  PSUM accumulates over all
  80 source tiles (3 bank regions of 512/512/226 fp32 columns); the result is
  DMAed out transposed and fixed up on host.
- No per-edge DMA descriptors at all: the only HBM traffic is the contiguous
  fp8 count matrix (12.8 MB/core), x (2.6 MB replicated), and the output.
  PE does 80x3 matmuls of K=128, N<=512 per core overlapping the M stream;
  dummy warm-up matmuls during the initial DMA fill keep the HAM clock warm.
  Measured ~57-62 us/iter on HW vs 492 us for the fp32 gather baseline.

Fallback strategy (KMODE=gather): node-sharded CSR dma_gather + DVE tree
reduction (see _build_program_gather) — bandwidth-bound at ~256B/edge.
"""

import os
from contextlib import ExitStack, nullcontext

import numpy as np

N = 10000
F = 128
E = 640000
NCORES = 8
P = 128

# mm-mode geometry
ST = 79                # source tiles of 128 nodes (79*128 = 10112 >= 10000)
NPAD = ST * P          # 10112 padded nodes (sources)
DLOC = N // NCORES     # 1250 destination nodes per core (exact, no padding)

# gather-mode geometry
NT = 80
S = NT // NCORES

_PROG_CACHE = {}


def _gdtype():
    return os.environ.get("KDTYPE", "f32")


# ---------------------------------------------------------------- mm mode ---


def _prep_mm(x, edge_index):
    import ml_dtypes

    x = np.ascontiguousarray(np.asarray(x, dtype=np.float32))
    ei = np.asarray(edge_index)
    row = ei[0].astype(np.int64)  # destination
    col = ei[1].astype(np.int64)  # source

    xpad = np.zeros((NPAD, F), np.float32)
    xpad[:N] = x
    # stationary layout: xsb[p, s, f] = x[s*128 + p, f]
    xsb = np.ascontiguousarray(
        xpad.reshape(ST, P, F).transpose(1, 0, 2)
    ).astype(ml_dtypes.bfloat16)

    msbs = []
    for c in range(NCORES):
        mask = (row >= c * DLOC) & (row < (c + 1) * DLOC)
        Mc = np.zeros((NPAD, DLOC), np.uint8)
        np.add.at(Mc, (col[mask], row[mask] - c * DLOC), 1)
        assert Mc.max() <= 16, "edge multiplicity too large for exact fp8"
        # moving layout: msb[p, s, d] = Mc[s*128 + p, d]
        msb = np.ascontiguousarray(
            Mc.reshape(ST, P, DLOC).transpose(1, 0, 2)
        ).astype(ml_dtypes.float8_e4m3)
        msbs.append(msb)
    return xsb, msbs


def _build_program_mm():
    import concourse.bass as bass  # noqa: F401
    import concourse.tile as tile
    from concourse import bacc, mybir

    nc = bacc.Bacc(
        "TRN2",
        target_bir_lowering=False,
        debug=False,
        num_devices=NCORES,
    )
    xp = nc.declare_dram_parameter("xp", [P, ST, F], mybir.dt.bfloat16, isOutput=False)
    m = nc.declare_dram_parameter("m", [P, ST, DLOC], mybir.dt.float8e4, isOutput=False)
    out = nc.declare_dram_parameter("out", [P, DLOC], mybir.dt.float32, isOutput=True)

    G = int(os.environ.get("KGROUP", "8"))     # source tiles per M-stream chunk
    _BUFS = int(os.environ.get("KBUFS", "4"))  # M-chunk buffers in flight
    _REPS = int(os.environ.get("KREPS", "1"))
    # chunk bounds over the ST source tiles (last chunk may be short)
    gbounds = list(range(0, ST, G)) + [ST]
    # psum column regions (each within one 2KB bank)
    spans = [(j * 512, min((j + 1) * 512, DLOC)) for j in range((DLOC + 511) // 512)]

    with tile.TileContext(nc) as tc:
        with ExitStack() as ctx:
            xpool = ctx.enter_context(tc.tile_pool(name="xs", bufs=1))
            mpool = ctx.enter_context(tc.tile_pool(name="mchunk", bufs=_BUFS))
            pspool = ctx.enter_context(tc.tile_pool(name="acc", bufs=1, space="PSUM"))

            xs = xpool.tile([P, ST, F], mybir.dt.bfloat16)
            if _REPS > 1:
                nc.sync.dma_start(xs[:], xp[:, :, :])
            opool = ctx.enter_context(tc.tile_pool(name="ostage", bufs=1))

            ps = [
                pspool.tile(
                    [P, n1 - n0], mybir.dt.float32, tag=f"ps{j}", name=f"ps{j}"
                )
                for j, (n0, n1) in enumerate(spans)
            ]

            # HAM warm-up: dummy matmuls keep the PE busy during the initial
            # DMA fill so the real matmul stream starts at the warm clock
            WARM = int(os.environ.get("KWARM", "14"))
            if WARM and _REPS == 1:
                wz = xpool.tile([P, 640], mybir.dt.bfloat16, name="warmz")
                nc.any.memzero(wz[:])
                wps = pspool.tile([P, 512], mybir.dt.float32, name="warmps")
                for _ in range(WARM):
                    nc.tensor.matmul(
                        wps[:], lhsT=wz[:, :128], rhs=wz[:, 128:640],
                        start=True, stop=True,
                    )

            loop_ctx = tc.For_i(0, _REPS, 1) if _REPS > 1 else nullcontext()
            with loop_ctx:
                for g0, g1 in zip(gbounds[:-1], gbounds[1:]):
                    gw = g1 - g0
                    if _REPS == 1:
                        # interleave the x chunk loads with the M stream so the
                        # first matmuls start after ~one chunk instead of the
                        # whole x load
                        nc.sync.dma_start(xs[:, g0:g1, :], xp[:, g0:g1, :])
                    mt = mpool.tile([P, G, DLOC], mybir.dt.float8e4, tag="m")
                    nc.sync.dma_start(mt[:, :gw, :], m[:, g0:g1, :])
                    for k in range(gw):
                        s = g0 + k
                        for j, (n0, n1) in enumerate(spans):
                            nc.tensor.matmul(
                                ps[j][:, :],
                                lhsT=xs[:, s, :],
                                rhs=mt[:, k, n0:n1],
                                start=(s == 0),
                                stop=(s == ST - 1),
                            )
                ostage = opool.tile([P, DLOC], mybir.dt.float32, tag="ostage")
                for j, (n0, n1) in enumerate(spans):
                    nc.vector.tensor_copy(ostage[:, n0:n1], ps[j][:, :])
                nc.sync.dma_start(out[:, :], ostage[:, :])

    nc.finalize()
    return nc


def _run_mm(x, edge_index):
    from concourse.bass_utils import run_bass_kernel_spmd

    xsb, msbs = _prep_mm(x, edge_index)

    key = (
        "mm",
        tuple(os.environ.get(k, "") for k in ("KGROUP", "KBUFS", "KREPS", "KWARM")),
    )
    if key not in _PROG_CACHE:
        _PROG_CACHE[key] = _build_program_mm()
    nc = _PROG_CACHE[key]

    in_maps = [{"xp": xsb, "m": msbs[c]} for c in range(NCORES)]
    res = run_bass_kernel_spmd(nc, in_maps, list(range(NCORES)))
    global LAST_RESULT
    LAST_RESULT = res

    h = np.zeros((N, F), dtype=np.float32)
    for c in range(NCORES):
        oc = np.asarray(res.results[c]["out"])  # [F, DLOC]
        lo = c * DLOC
        hi = min((c + 1) * DLOC, N)
        if hi > lo:
            h[lo:hi] = oc[:, : hi - lo].T
    return h


# ------------------------------------------------------------ gather mode ---


def _prep_gather(x, edge_index):
    x = np.ascontiguousarray(np.asarray(x, dtype=np.float32))
    ei = np.asarray(edge_index)
    row = ei[0].astype(np.int64)
    col = ei[1].astype(np.int64)

    deg = np.bincount(row, minlength=N)

    # nodes ordered by degree desc; stable for reproducibility
    order = np.argsort(-deg, kind="stable")
    order_pad = np.full(NT * P, -1, dtype=np.int64)
    order_pad[:N] = order

    # CSR of incoming neighbors, grouped by destination row
    eorder = np.argsort(row, kind="stable")
    scol = col[eorder].astype(np.int16)
    indptr = np.zeros(N + 1, dtype=np.int64)
    indptr[1:] = np.cumsum(deg)

    # slot max degrees: tiles 8s..8s+7 in slot s; degrees are non-increasing
    # along order_pad so the first node of tile 8s has the slot max.
    Ks = []
    degs_sorted = np.zeros(NT * P, dtype=np.int64)
    degs_sorted[:N] = deg[order]
    for s in range(S):
        Ks.append(max(int(degs_sorted[(s * NCORES) * P]), 1))

    # x with an extra zero row used by padding indices
    if _gdtype() == "bf16":
        import ml_dtypes

        xpad = np.zeros((N + 1, F), dtype=ml_dtypes.bfloat16)
        xpad[:N] = x.astype(ml_dtypes.bfloat16)
    else:
        xpad = np.zeros((N + 1, F), dtype=np.float32)
        xpad[:N] = x

    # per-core wrapped index tensors
    idx_cores = []
    for c in range(NCORES):
        blocks = []
        for s in range(S):
            K = Ks[s]
            t = s * NCORES + c
            blk = np.full((K, P), N, dtype=np.int16)
            for p in range(P):
                v = order_pad[t * P + p]
                if v >= 0:
                    d0, d1 = indptr[v], indptr[v + 1]
                    if d1 > d0:
                        blk[: d1 - d0, p] = scol[d0:d1]
            idx_lin = blk.reshape(-1)            # i = j*128 + p
            w = idx_lin.reshape(-1, 16)          # [n/16, 16]
            sb = np.tile(w.T, (8, 1))            # [128, n/16] replicated x8
            blocks.append(sb)
        idx_cores.append(np.ascontiguousarray(np.concatenate(blocks, axis=1)))

    return xpad, idx_cores, Ks, order_pad


def _build_program_gather(Ks):
    import concourse.bass as bass  # noqa: F401
    import concourse.tile as tile
    from concourse import bacc, mybir

    s_total = sum(K * P // 16 for K in Ks)

    nc = bacc.Bacc(
        "TRN2",
        target_bir_lowering=False,
        debug=False,
        num_devices=NCORES,
        dynamic_dma_scratch_size=int(os.environ.get("KSCRATCH", "98304")),
        num_swdge_queues=int(os.environ.get("KNQ", "4")),
    )
    gdt = mybir.dt.bfloat16 if _gdtype() == "bf16" else mybir.dt.float32
    xp = nc.declare_dram_parameter("xp", [N + 1, F], gdt, isOutput=False)
    idx = nc.declare_dram_parameter("idx", [128, s_total], mybir.dt.int16, isOutput=False)
    out = nc.declare_dram_parameter("out", [S * P, F], mybir.dt.float32, isOutput=True)

    CHUNK = int(os.environ.get("KCHUNK", "12"))  # blocks of 128 idx per dma_gather

    with tile.TileContext(nc) as tc:
        with ExitStack() as ctx:
            _BUFS = int(os.environ.get("KBUFS", "10"))
            _REPS = int(os.environ.get("KREPS", "1"))
            ipool = ctx.enter_context(tc.tile_pool(name="idx", bufs=1))
            gpool = ctx.enter_context(tc.tile_pool(name="gath", bufs=_BUFS))
            apool = ctx.enter_context(tc.tile_pool(name="acc", bufs=min(_BUFS, 2)))

            # load the whole wrapped index array into SBUF once
            it_all = ipool.tile([128, s_total], mybir.dt.int16)
            nc.sync.dma_start(it_all[:], idx[:, :])

            _NQ = int(os.environ.get("KNQ", "4"))
            _gq = [0]

            loop_ctx = tc.For_i(0, _REPS, 1) if _REPS > 1 else nullcontext()
            with loop_ctx:
                off = 0  # col offset into idx (wrapped: block j -> cols j*8..j*8+7)
                for s in range(S):
                    K = Ks[s]
                    acc = apool.tile([128, F], mybir.dt.float32, tag="acc")
                    nchunks = (K + CHUNK - 1) // CHUNK
                    # equalize chunk sizes (avoid tiny tail gathers)
                    base, extra = divmod(K, nchunks)
                    bounds = [0]
                    for c in range(nchunks):
                        bounds.append(bounds[-1] + base + (1 if c < extra else 0))
                    for c in range(nchunks):
                        j0, j1 = bounds[c], bounds[c + 1]
                        W = j1 - j0
                        n = W * P
                        g = gpool.tile([128, n], gdt, tag="g")
                        nc.gpsimd.dma_gather(
                            g[:].rearrange("p (k f) -> p k f", f=F),
                            xp[:, :],
                            it_all[:, off + j0 * 8 : off + j1 * 8],
                            num_idxs=n,
                            num_idxs_reg=n,
                            elem_size=F,
                            single_packet=False,
                            queue_num=_gq[0],
                        )
                        _gq[0] = (_gq[0] + 1) % _NQ

                        # in-place tree reduction over the W feature blocks
                        while W > 1:
                            half = W // 2
                            nc.vector.tensor_add(
                                out=g[:, 0 : half * F],
                                in0=g[:, 0 : half * F],
                                in1=g[:, (W - half) * F : W * F],
                            )
                            W -= half
                        red = g
                        if c == 0:
                            nc.vector.tensor_copy(acc[:], red[:, 0:F])
                        else:
                            nc.vector.tensor_add(out=acc[:], in0=acc[:], in1=red[:, 0:F])

                    nc.sync.dma_start(out[s * P : (s + 1) * P, :], acc[:])
                    off += K * 8

    nc.finalize()
    return nc


def _run_gather(x, edge_index):
    from concourse.bass_utils import run_bass_kernel_spmd

    xpad, idx_cores, Ks, order_pad = _prep_gather(x, edge_index)

    key = (
        "gather",
        tuple(Ks),
        tuple(
            os.environ.get(k, "")
            for k in ("KCHUNK", "KBUFS", "KREPS", "KSCRATCH", "KNQ", "KDTYPE")
        ),
    )
    if key not in _PROG_CACHE:
        _PROG_CACHE[key] = _build_program_gather(Ks)
    nc = _PROG_CACHE[key]

    in_maps = [{"xp": xpad, "idx": idx_cores[c]} for c in range(NCORES)]
    try:
        res = run_bass_kernel_spmd(nc, in_maps, list(range(NCORES)))
    except Exception:
        # fall back to a conservative configuration (smaller gathers,
        # shallow pipelining) in case the tuned one trips the SWDGE ring
        os.environ["KCHUNK"] = "8"
        os.environ["KBUFS"] = "2"
        os.environ["KNQ"] = "1"
        os.environ["KSCRATCH"] = "16384"
        nc = _build_program_gather(Ks)
        res = run_bass_kernel_spmd(nc, in_maps, list(range(NCORES)))
    global LAST_RESULT
    LAST_RESULT = res

    h = np.zeros((N, F), dtype=np.float32)
    for c in range(NCORES):
        oc = np.asarray(res.results[c]["out"]).astype(np.float32)
        for s in range(S):
            t = s * NCORES + c
            vs = order_pad[t * P : (t + 1) * P]
            valid = vs >= 0
            if valid.any():
                h[vs[valid]] = oc[s * P : (s + 1) * P][valid]
    return h


def kernel(x, edge_index):
    mode = os.environ.get("KMODE", "mm")
    if mode == "gather":
        return _run_gather(x, edge_index)
    try:
        return _run_mm(x, edge_index)
    except Exception:
        # emergency path: fp32 CSR gather (slower but fully validated)
        return _run_gather(x, edge_index)
